# revision 1
# baseline (speedup 1.0000x reference)
"""Trainium2 Bass kernel for nn_GatFeatDecoder (GAT-style decoder).

Reference computation per batch b (B=16, W=64, K=256, E=128, O=64):
    v = x[b].T                               (K, W)
    l = v @ W1.T ; r = v @ W2.T              (K, E) each
    e[i,j]  = sum_e a_e * LeakyReLU(l[i,e] + r[j,e] + lin_b[e]) + bias_kk[i,j]
    attn    = softmax_j(e)
    h       = sigmoid(attn @ v)              (K, W)
    out[b]  = h.T @ fc_w.T + fc_b            (W, O)

Data-parallel: 2 batches per core on 8 cores, no collectives.

Math folding (per-core):
  * z~ = (1-a)|a_e| (l+r+b); sum_e a_e LeakyReLU = sum_e sgn_e relu(z~)
    + alpha' * sum_e sgn_e z~, alpha' = alpha/(1-alpha).  The per-i part
    of the linear term cancels in softmax; the per-j part srb_j =
    sum_w q_w xb[w,j] with q = alpha' * W2b @ sgn precomputed on host,
    and enters as the per-partition bias of the exp() activation.
  * bias_kk^T is accumulated into the score PSUM tile by one extra
    matmul with identity weights, so exp() reads PSUM directly.
  * relu tiles T^J[e,i] = relu(lt[e,i] + rtb[e,J]) are produced by a
    3-way DVE / ACT / Pool split (one instr per J).
  * score row J comes from a matmul whose [128,128] weight slice (from a
    sliding window of a sign-master matrix) has the sign vector at
    column j = J mod 128; 128 J-matmuls + the bias matmul form one PSUM
    accumulation chain per (batch, j-half).
  * softmax without row-max (logits bounded): P^T = exp(S^T + srb_j).
    attn@v and the denominator come from matmuls with rhs = [v | 2.0];
    h = sigmoid(num/den) = 0.5*(tanh(num * (0.5/den)) + 1) via ACT Tanh
    with per-partition scale = reciprocal(2*sum exp); the 0.5/0.5 affine
    is folded into the fc weights/bias on the host.
  * All constants + inputs arrive in two packed bf16 DMAs (packA via SP
    HWDGE, packB via Pool SWDGE, issued in parallel); both batches'
    outputs leave in one SP-HWDGE DMA.

Pipeline shaping:
  * PE warm-up: junk matmuls against a memset scratch tile run while the
    input DMA is in flight and bridge every gap until the first relu
    tile, so the whole real stream executes at the full 2.4 GHz pstate
    (the cost model drops to 1.2/0.65 GHz after any PE idle).
  * Batch-0's setup copies (lt, rtb) run on DVE so the first relu tiles
    (also DVE) follow in-order with no cross-engine semaphore -- Tile's
    sem aligner otherwise coarsens the wait onto a later ACT tick.
  * Emission is software-pipelined across the two batches (batch-1 tile
    generation is issued before batch-0's attn/fc epilogue), and the
    attn i-halves accumulate in different PSUM banks so the second half
    is not serialized behind the first half's readers.

Cost-model timeline: 65.8 us vs a ~56 us PE-streaming floor (the
O(K^2 E) relu-tile data must cross the PE once; fp8 DoubleRow would
halve that but breaks the producers' 4x DVE mode, a net loss).
"""

import numpy as np
import ml_dtypes

import concourse.bass as bass
import concourse.bacc as bacc
import concourse.tile as tile
from concourse import mybir
from concourse.bass_utils import run_bass_kernel_spmd

ALPHA = 0.2
B, Wn, K, E, O = 16, 64, 256, 128, 64
N_CORES = 8
BPC = B // N_CORES  # batches per core

FP32 = mybir.dt.float32
BF16 = mybir.dt.bfloat16
AF = mybir.ActivationFunctionType
ALU = mybir.AluOpType

# ---- packed-constant column layout (bf16, 128 partitions) ----
# pack A (early: needed for batch-0 projections + first relu tiles)
A_W1 = 0                      # w1at   [64,128]  rows 0:64
A_W2 = A_W1 + E               # w2bt   [65,128]  rows 0:65
A_XB0 = A_W2 + E              # xb b0  [65,256]  rows 0:65 (row 64 = ones)
A_Q = A_XB0 + K               # q      [65,1]
A_XB1 = A_Q + 1               # xb b1  [65,256]
A_COLS = A_XB1 + K

# pack B (needed from ~4.5us in; lands by ~4us)
B_SS = 0                      # sign master [128,256] (col 128 = sgn)
B_XTO = B_SS + 256            # xto2   4 x [128,65]  (b,h) = [v | 2.0]
B_BKT = B_XTO + 4 * (Wn + 1)  # bkkt^T 2 x [128,256]
B_ID = B_BKT + 2 * K          # identity [128,128]
B_FCW = B_ID + E              # fcw2t  2 x [128,64]
B_FCB = B_FCW + 2 * O         # fcb2   [64,1]
B_COLS = B_FCB + 1

# tile-gen engine split: per j-half index idx in 0..127
# idx%16 in {3,8,13} -> Pool, {5,10,15} -> ACT, else DVE
POOL_SET = {3, 8, 13}
ACT_SET = {5, 10, 15}


def _build_program():
    nc = bacc.Bacc("TRN2", target_bir_lowering=False, debug=False,
                   num_devices=N_CORES)

    d_packA = nc.dram_tensor("packA", [128, A_COLS], BF16, kind="ExternalInput")
    d_packB = nc.dram_tensor("packB", [128, B_COLS], BF16, kind="ExternalInput")
    d_out = nc.dram_tensor("outp", [O, BPC * Wn], FP32, kind="ExternalOutput")

    with tile.TileContext(nc) as tc:
        with (
            tc.tile_pool(name="consts", bufs=1) as consts,
            tc.tile_pool(name="setup", bufs=2) as setup,
            tc.tile_pool(name="trelu", bufs=32) as trelu,
            tc.tile_pool(name="etiles", bufs=4) as etiles,
            tc.tile_pool(name="small", bufs=8) as small,
            tc.tile_pool(name="psA", bufs=2, space="PSUM") as psA,
            tc.tile_pool(name="psS", bufs=2, space="PSUM") as psS,
            tc.tile_pool(name="psM", bufs=2, space="PSUM") as psM,
            tc.tile_pool(name="psN", bufs=2, space="PSUM") as psN,
        ):
            # scratch for PE warm-up matmuls (burns through the pstate ramp
            # while the input DMA is in flight)
            warm = consts.tile([128, 512], BF16, tag="warm")
            nc.gpsimd.memset(warm[:], 0.0)

            cA = consts.tile([128, A_COLS], BF16, tag="cA")
            nc.sync.dma_start(out=cA[:, 0:A_XB1], in_=d_packA.ap()[:, 0:A_XB1])
            nc.sync.dma_start(out=cA[:, A_XB1:A_COLS],
                              in_=d_packA.ap()[:, A_XB1:A_COLS])
            cB = consts.tile([128, B_COLS], BF16, tag="cB")
            nc.gpsimd.dma_start(out=cB[:], in_=d_packB.ap())

            w1at_v = cA[0:Wn, A_W1:A_W1 + E]
            w2bt_v = cA[0:Wn + 1, A_W2:A_W2 + E]
            q_v = cA[0:Wn + 1, A_Q:A_Q + 1]

            def xb_v(b):
                c = A_XB0 if b == 0 else A_XB1
                return cA[0:Wn + 1, c:c + K]

            def ss_v(j):  # [128,128] weights, sgn at col j
                return cB[:, B_SS + 128 - j:B_SS + 256 - j]

            def xto_v(b, h):
                c = B_XTO + (2 * b + h) * (Wn + 1)
                return cB[:, c:c + Wn + 1]

            def bkt_v(jh):
                return cB[:, B_BKT + jh * K:B_BKT + (jh + 1) * K]

            ident_v = cB[:, B_ID:B_ID + E]

            def fcw_v(ih):
                return cB[:, B_FCW + ih * O:B_FCW + (ih + 1) * O]

            fcb_v = cB[0:O, B_FCB:B_FCB + 1]

            # per-batch state
            lt_b = [None] * BPC
            rtb_f = [None] * BPC
            srb_f = [None] * BPC
            pT = [[None, None] for _ in range(BPC)]
            tt = [[None, None] for _ in range(BPC)]
            ps_lr_t = [None] * BPC
            ps_mix_t = [None] * BPC   # [:,0:65] attn ih0 | [:,66:68] srb | [0:64,68:132] fc

            ot2 = consts.tile([O, BPC * Wn], FP32, tag="ot2")

            # PE warm-up: ~3us of junk matmuls so the real stream runs at
            # full clock from the first instruction
            ps_lr_t[0] = psA.tile([E, 2 * K], FP32, tag="ps_lr", name="ps_lr0")
            for w in (512, 512, 512, 512, 256, 128):
                nc.tensor.matmul(ps_lr_t[0][:, 0:w], warm[:, 0:128],
                                 warm[:, 0:w], start=True, stop=True,
                                 skip_group_check=True)

            def emit_proj(b):
                if ps_lr_t[b] is None:
                    ps_lr_t[b] = psA.tile([E, 2 * K], FP32, tag="ps_lr",
                                          name=f"ps_lr{b}")
                ps_lr = ps_lr_t[b]
                nc.tensor.matmul(ps_lr[:, 0:K], w1at_v, xb_v(b)[0:Wn, :],
                                 start=True, stop=True)
                nc.tensor.matmul(ps_lr[:, K:2 * K], w2bt_v, xb_v(b),
                                 start=True, stop=True)
                # lt (bf16) and rtb (fp32, scalar-ptr source).  For batch 0
                # both copies run on DVE so the first relu tiles (also DVE)
                # follow in-order with no cross-engine semaphore.
                lt_b[b] = setup.tile([E, K], BF16, tag="lt_b", name=f"lt_b{b}")
                rtb_f[b] = setup.tile([E, K], FP32, tag="rtb_f", name=f"rtb_f{b}")
                if b == 0:
                    # chunked rtb copy: only the first 32 bias columns gate
                    # tile j0; the rest stream between tiles (emit_gen)
                    nc.vector.tensor_copy(lt_b[b][:], ps_lr[:, 0:K])
                    nc.vector.tensor_copy(rtb_f[b][:, 0:32], ps_lr[:, K:K + 32])
                else:
                    nc.scalar.copy(lt_b[b][:], ps_lr[:, 0:K])
                    nc.scalar.copy(rtb_f[b][:], ps_lr[:, K:2 * K])
                # srb columns: srb[j] = sum_w q[w] xb[w, j]
                ps_mix_t[b] = psM.tile([128, 132], FP32, tag="ps_mix",
                                       name=f"ps_mix{b}")
                for jh in range(2):
                    nc.tensor.matmul(ps_mix_t[b][:, 66 + jh:67 + jh],
                                     xb_v(b)[:, 128 * jh:128 * jh + 128],
                                     q_v, start=True, stop=True)
                srb_f[b] = setup.tile([K // 2, 2], FP32, tag="srb_f",
                                      name=f"srb_f{b}")
                nc.scalar.copy(srb_f[b][:], ps_mix_t[b][:, 66:68])

            def emit_gen(b, jh):
                ps_sc = psS.tile([K // 2, K], FP32, tag="ps_sc")
                if (b, jh) == (0, 0):
                    # bridge the gap between the projections and the first
                    # relu tile so the PE ramp never resets
                    for _ in range(4):
                        nc.tensor.matmul(ps_sc[:], warm[:, 0:128],
                                         warm[:, 0:256], start=True,
                                         stop=True, skip_group_check=True)
                # open the chain with bias_kk^T via identity weights (off
                # the critical tail: exp depends only on the last relu mm)
                nc.tensor.matmul(ps_sc[:], ident_v, bkt_v(jh),
                                 start=True, stop=False,
                                 skip_group_check=True)
                warm_chunk = (b, jh) == (0, 0)
                lt_v = lt_b[b][:]
                for j in range(128):
                    J = 128 * jh + j
                    idx = j % 16
                    # warm-up chunk: ACT/Pool are still busy with the
                    # setup copies, DVE carries the first tiles
                    act_set, pool_set = ACT_SET, POOL_SET
                    if warm_chunk and j < 16:
                        act_set, pool_set = {15}, {8, 13}
                    bias_c = rtb_f[b][:, J:J + 1]
                    tj = trelu.tile([E, K], BF16, tag="tj")
                    if idx in act_set:
                        nc.scalar.activation(
                            tj[:], lt_v, AF.Relu, bias=bias_c, scale=1.0)
                    elif idx in pool_set:
                        nc.gpsimd.tensor_scalar(
                            out=tj[:], in0=lt_v,
                            scalar1=bias_c, scalar2=0.0,
                            op0=ALU.add, op1=ALU.max)
                    else:
                        nc.vector.tensor_scalar(
                            out=tj[:], in0=lt_v,
                            scalar1=bias_c, scalar2=0.0,
                            op0=ALU.add, op1=ALU.max)
                    nc.tensor.matmul(ps_sc[:], ss_v(j), tj[:],
                                     start=False, stop=(j == 127),
                                     skip_group_check=True)
                    if warm_chunk:
                        # stream the remaining rtb chunks in with the tiles
                        if j in (16, 48, 80):
                            c = (j + 16) // 32
                            nc.vector.tensor_copy(
                                rtb_f[b][:, 32 * c:32 * c + 32],
                                ps_lr_t[0][:, K + 32 * c:K + 32 * c + 32])
                        elif j == 112:
                            nc.vector.tensor_copy(
                                rtb_f[b][:, 128:K],
                                ps_lr_t[0][:, K + 128:2 * K])
                pT[b][jh] = etiles.tile([K // 2, K], BF16, tag="pT",
                                        name=f"pT{b}_{jh}")
                nc.scalar.activation(pT[b][jh][:], ps_sc[:], AF.Exp,
                                     bias=srb_f[b][:, jh:jh + 1], scale=1.0)

            def emit_attn(b):
                ps_att1 = psN.tile([K // 2, Wn + 1], FP32, tag="ps_att1",
                                   name=f"ps_att1{b}")
                nums = [ps_mix_t[b][:, 0:Wn + 1], ps_att1]
                for ih in range(2):
                    # separate PSUM banks per i-half so the ih1 chain is not
                    # serialized behind ih0's readers
                    for jh in range(2):
                        nc.tensor.matmul(
                            nums[ih], pT[b][jh][:, 128 * ih:128 * ih + 128],
                            xto_v(b, jh), start=(jh == 0), stop=(jh == 1))
                    rcol = small.tile([K // 2, 1], FP32, tag=f"rcol{ih}",
                                      name=f"rcol{b}_{ih}")
                    nc.vector.reciprocal(rcol[:], nums[ih][:, Wn:Wn + 1])
                    tt[b][ih] = small.tile([K // 2, Wn], BF16, tag=f"tt{ih}",
                                           name=f"tt{b}_{ih}")
                    nc.scalar.activation(tt[b][ih][:], nums[ih][:, 0:Wn],
                                         AF.Tanh, scale=rcol[:])

            def emit_fc_mm(b):
                ps_o = ps_mix_t[b][0:O, 68:68 + Wn]
                for ih in range(2):
                    nc.tensor.matmul(ps_o, fcw_v(ih), tt[b][ih][:],
                                     start=(ih == 0), stop=(ih == 1))

            def emit_fc_out(b):
                nc.scalar.activation(ot2[:, Wn * b:Wn * (b + 1)],
                                     ps_mix_t[b][0:O, 68:68 + Wn],
                                     AF.Identity, bias=fcb_v)

            # software-pipelined emission
            emit_proj(0)
            emit_proj(1)
            emit_gen(0, 0)
            emit_gen(0, 1)
            emit_gen(1, 0)
            emit_attn(0)
            emit_gen(1, 1)
            emit_fc_mm(0)
            emit_attn(1)
            emit_fc_mm(1)
            emit_fc_out(0)
            emit_fc_out(1)
            nc.sync.dma_start(out=d_out.ap(), in_=ot2[:])

    nc.compile()
    return nc



_NC_CACHE = {}


def _get_program():
    if "nc" not in _NC_CACHE:
        _NC_CACHE["nc"] = _build_program()
    return _NC_CACHE["nc"]


def _host_prep(x, lin_w, lin_b, a, bias_kk, fc_w, fc_b):
    f32 = np.float32
    bf16 = ml_dtypes.bfloat16
    x = np.ascontiguousarray(x, f32)
    aa = (np.abs(a) * (1.0 - ALPHA)).astype(f32)
    sgn = np.sign(a).astype(f32)
    alpha_p = ALPHA / (1.0 - ALPHA)

    w1at = (lin_w[:, :Wn] * aa[:, None]).T.astype(f32)          # [64,128]
    w2t = (lin_w[:, Wn:] * aa[:, None]).T                        # [64,128]
    bt = (lin_b * aa)[None, :]
    w2bt = np.concatenate([w2t, bt], 0).astype(f32)              # [65,128]
    q = (alpha_p * (w2bt @ sgn)).astype(f32)                     # [65]
    bkkt = bias_kk.T.astype(f32)                                 # [256,256]
    fcw2t = (0.5 * fc_w).T.astype(f32)                           # [256,64]
    fcb2 = (fc_b + 0.5 * fc_w.sum(1)).astype(f32)                # [64]

    packA = np.zeros((128, A_COLS), f32)
    packA[0:Wn, A_W1:A_W1 + E] = w1at
    packA[0:Wn + 1, A_W2:A_W2 + E] = w2bt
    packA[0:Wn + 1, A_Q] = q


    packB_shared = np.zeros((128, B_COLS), f32)
    packB_shared[:, B_SS + 128] = sgn
    packB_shared[:, B_BKT:B_BKT + K] = bkkt[0:128, :]
    packB_shared[:, B_BKT + K:B_BKT + 2 * K] = bkkt[128:256, :]
    packB_shared[:, B_ID:B_ID + E] = np.eye(128, dtype=f32)
    packB_shared[:, B_FCW:B_FCW + O] = fcw2t[0:128, :]
    packB_shared[:, B_FCW + O:B_FCW + 2 * O] = fcw2t[128:256, :]
    packB_shared[0:O, B_FCB] = fcb2

    in_maps = []
    for c in range(N_CORES):
        pa = packA.copy()
        pb = packB_shared.copy()
        for i in range(BPC):
            xb = x[BPC * c + i]                                  # [64,256]
            xb1 = np.concatenate([xb, np.ones((1, K), f32)], 0)  # [65,256]
            vt = xb.T                                            # [256,64]
            xto2 = np.concatenate([vt, np.full((K, 1), 2.0, f32)], 1)
            pa[0:Wn + 1, (A_XB0 if i == 0 else A_XB1):(A_XB0 if i == 0 else A_XB1) + K] = xb1
            for h in range(2):
                c0 = B_XTO + (2 * i + h) * (Wn + 1)
                pb[:, c0:c0 + Wn + 1] = xto2[128 * h:128 * h + 128, :]
        in_maps.append({
            "packA": np.ascontiguousarray(pa.astype(bf16)),
            "packB": np.ascontiguousarray(pb.astype(bf16)),
        })
    return in_maps


def kernel(x, lin_w, lin_b, a, bias_kk, fc_w, fc_b, _trace=False):
    nc = _get_program()
    in_maps = _host_prep(np.asarray(x), np.asarray(lin_w), np.asarray(lin_b),
                         np.asarray(a), np.asarray(bias_kk),
                         np.asarray(fc_w), np.asarray(fc_b))
    res = run_bass_kernel_spmd(nc, in_maps, list(range(N_CORES)),
                               trace=_trace)
    out = np.empty((B, Wn, O), np.float32)
    for c in range(N_CORES):
        o = res.results[c]["outp"]          # (O, BPC*Wn)
        for i in range(BPC):
            out[BPC * c + i] = o[:, Wn * i:Wn * (i + 1)].T
    if _trace:
        return out, res
    return out



# revision 11
# speedup vs baseline: 1.2059x; 1.2059x over previous
"""Trainium2 Bass kernel for nn_GatFeatDecoder (GAT-style decoder).

Reference computation per batch b (B=16, W=64, K=256, E=128, O=64):
    v = x[b].T                               (K, W)
    l = v @ W1.T ; r = v @ W2.T              (K, E) each
    e[i,j]  = sum_e a_e * LeakyReLU(l[i,e] + r[j,e] + lin_b[e]) + bias_kk[i,j]
    attn    = softmax_j(e)
    h       = sigmoid(attn @ v)              (K, W)
    out[b]  = h.T @ fc_w.T + fc_b            (W, O)

Data-parallel: 2 batches per core on 8 cores, no collectives.

Math folding (per-core), same as the v1 kernel:
  * z~ = (1-a)|a_e| (l+r+b); sum_e a_e LeakyReLU = sum_e sgn_e relu(z~)
    + alpha' * sum_e sgn_e z~, alpha' = alpha/(1-alpha).  The per-i part
    of the linear term cancels in softmax; the per-j part srb_j =
    sum_w q_w xb[w,j] with q = alpha' * W2b @ sgn precomputed on device,
    and enters as the per-partition bias of the exp() activation.
  * bias_kk^T is accumulated into each score PSUM tile by one extra
    matmul with identity weights, so exp() reads PSUM directly.
  * softmax without row-max (logits bounded): P^T = exp(S^T + srb_j).
    attn@v and the denominator come from matmuls with rhs = [v | 2.0];
    h = sigmoid(num/den) = 0.5*(tanh(num * (0.5/den)) + 1) via ACT Tanh
    with per-partition scale = reciprocal(2*sum exp); the 0.5/0.5 affine
    is folded into the fc weights/bias on the host.

v2 score-matmul inversion (the big change vs v1):
  * v1 streamed each relu tile [E,K] through the PE as the MOVING
    operand of a sign-weighted matmul (256 rows -> ~107ns each, a
    54.6us PE floor for 512 tiles).  v2 makes the relu tile the
    STATIONARY operand instead: per query-node i the tile
    T^i[e, j] = relu(rtb[e,j] + lt[e,i]) is produced once, and two
    matmuls (one per j-half chain) contract it against a single sgn
    column as the moving operand, writing one PSUM column
    S^T[jh][:, i].  Output free size is 1, so each matmul costs ~4ns;
    the whole score reduction is ~2us of PE time and the kernel is
    bound by relu-tile PRODUCTION on DVE/ACT/Pool instead
    (134/391/429 ns per [128,256] tile; DVE runs in 4x mode on bf16).
  * Tiles are indexed by i (bias = lt column) rather than j (bias =
    rtb column) so S^T lands in the same [j-half, i] layout v1 used;
    the exp/attn/fc epilogue is unchanged.
  * The PE is now nearly idle, so the p-state ramp is irrelevant and
    v1's warm-up matmul prologue is dropped entirely.
  * Production is split DVE/ACT/Pool ~20/6/6 per 32 tiles (inverse to
    the measured per-tile costs), with the fixed per-engine work (exp
    on ACT, copies on DVE/Pool) folded into the balance.
"""

import numpy as np
import ml_dtypes

import concourse.bass as bass
import concourse.bacc as bacc
import concourse.tile as tile
from concourse import mybir
from concourse.bass_utils import run_bass_kernel_spmd

ALPHA = 0.2
B, Wn, K, E, O = 16, 64, 256, 128, 64
N_CORES = 8
BPC = B // N_CORES  # batches per core

FP32 = mybir.dt.float32
BF16 = mybir.dt.bfloat16
AF = mybir.ActivationFunctionType
ALU = mybir.AluOpType

# ---- packed-constant column layout (bf16, 128 partitions) ----
# pack A (early: needed for projections + first relu tiles)
A_W1 = 0                      # w1at   [64,128]  rows 0:64
A_W2 = A_W1 + E               # w2bt   [65,128]  rows 0:65
A_Q = A_W2 + E                # q      [65,1]
A_XB0 = A_Q + 1               # xb b0  [65,256]  rows 0:65 (row 64 = ones)
A_XB1 = A_XB0 + K             # xb b1  [65,256]
A_COLS = A_XB1 + K

# pack B (epilogue constants; lands while batch-0 tiles stream)
B_SGN = 0                     # sgn column [128,1]
B_XTO = B_SGN + 1             # xto2   4 x [128,65]  (b,h) = [v | 2.0]
B_BKT = B_XTO + 4 * (Wn + 1)  # bkkt^T 2 x [128,256]
B_ID = B_BKT + 2 * K          # identity [128,128]
B_FCW = B_ID + E              # fcw2t  2 x [128,64]
B_FCB = B_FCW + 2 * O         # fcb2   [64,1]
B_COLS = B_FCB + 1

# tile-production engine split per 32 i's: DVE 20, ACT 6, Pool 6
# (largest-remainder interleave of the measured per-tile costs)
def _make_pattern(n_v=20, n_a=6, n_p=6):
    quota = {"V": n_v / 32.0, "A": n_a / 32.0, "P": n_p / 32.0}
    acc = {"V": 0.0, "A": 0.0, "P": 0.0}
    pat = []
    for _ in range(32):
        for k in acc:
            acc[k] += quota[k]
        k = max(acc, key=lambda t: acc[t])
        acc[k] -= 1.0
        pat.append(k)
    return pat

PATTERN = _make_pattern()


def _build_program():
    nc = bacc.Bacc("TRN2", target_bir_lowering=False, debug=False,
                   num_devices=N_CORES)

    d_packA = nc.dram_tensor("packA", [128, A_COLS], BF16, kind="ExternalInput")
    d_packB = nc.dram_tensor("packB", [128, B_COLS], BF16, kind="ExternalInput")
    d_out = nc.dram_tensor("outp", [O, BPC * Wn], FP32, kind="ExternalOutput")

    with tile.TileContext(nc) as tc:
        with (
            tc.tile_pool(name="consts", bufs=1) as consts,
            tc.tile_pool(name="setup", bufs=2) as setup,
            tc.tile_pool(name="trelu", bufs=24) as trelu,
            tc.tile_pool(name="etiles", bufs=4) as etiles,
            tc.tile_pool(name="small", bufs=8) as small,
            tc.tile_pool(name="psA", bufs=2, space="PSUM") as psA,
            tc.tile_pool(name="psS", bufs=2, space="PSUM") as psS,
            tc.tile_pool(name="psM", bufs=2, space="PSUM") as psM,
            tc.tile_pool(name="psN", bufs=2, space="PSUM") as psN,
        ):
            cA = consts.tile([128, A_COLS], BF16, tag="cA")
            # chunk 1: weights + q + xb0 (gates batch-0 projections);
            # chunk 2: xb1
            nc.sync.dma_start(out=cA[:, 0:A_XB1], in_=d_packA.ap()[:, 0:A_XB1])
            nc.sync.dma_start(out=cA[:, A_XB1:A_COLS],
                              in_=d_packA.ap()[:, A_XB1:A_COLS])
            cB = consts.tile([128, B_COLS], BF16, tag="cB")
            nc.gpsimd.dma_start(out=cB[:], in_=d_packB.ap())

            w1at_v = cA[0:Wn, A_W1:A_W1 + E]
            w2bt_v = cA[0:Wn + 1, A_W2:A_W2 + E]
            q_v = cA[0:Wn + 1, A_Q:A_Q + 1]

            def xb_v(b):
                c = A_XB0 if b == 0 else A_XB1
                return cA[0:Wn + 1, c:c + K]

            sgn_v = cB[:, B_SGN:B_SGN + 1]

            def xto_v(b, h):
                c = B_XTO + (2 * b + h) * (Wn + 1)
                return cB[:, c:c + Wn + 1]

            def bkt_v(jh):
                return cB[:, B_BKT + jh * K:B_BKT + (jh + 1) * K]

            ident_v = cB[:, B_ID:B_ID + E]

            def fcw_v(ih):
                return cB[:, B_FCW + ih * O:B_FCW + (ih + 1) * O]

            fcb_v = cB[0:O, B_FCB:B_FCB + 1]

            # per-batch state
            lt_f = [None] * BPC     # fp32 [E,K]  (scalar/bias source, per-i)
            rtb_b = [None] * BPC    # bf16 [E,K]  (tile in0)
            srb_f = [None] * BPC
            ps_sc = [None] * BPC    # pair of [K//2, K] score PSUM tiles
            pT = [[None, None] for _ in range(BPC)]
            tt = [[None, None] for _ in range(BPC)]
            ps_mix_t = [None] * BPC  # [:,0:65] attn ih0 | [:,66:68] srb | [0:64,68:132] fc

            ot2 = consts.tile([O, BPC * Wn], FP32, tag="ot2")

            def emit_proj(b):
                ps_lr = psA.tile([E, 2 * K], FP32, tag="ps_lr",
                                 name=f"ps_lr{b}")
                nc.tensor.matmul(ps_lr[:, 0:K], w1at_v, xb_v(b)[0:Wn, :],
                                 start=True, stop=True)
                nc.tensor.matmul(ps_lr[:, K:2 * K], w2bt_v, xb_v(b),
                                 start=True, stop=True)
                lt_f[b] = setup.tile([E, K], FP32, tag="lt_f", name=f"lt_f{b}")
                rtb_b[b] = setup.tile([E, K], BF16, tag="rtb_b",
                                      name=f"rtb_b{b}")
                # both copies on DVE: rtb feeds the first DVE tiles
                # in-order; Pool cannot read PSUM
                nc.vector.tensor_copy(rtb_b[b][:], ps_lr[:, K:2 * K])
                nc.vector.tensor_copy(lt_f[b][:], ps_lr[:, 0:K])
                # srb columns: srb[j] = sum_w q[w] xb[w, j]
                ps_mix_t[b] = psM.tile([128, 132], FP32, tag="ps_mix",
                                       name=f"ps_mix{b}")
                for jh in range(2):
                    nc.tensor.matmul(ps_mix_t[b][:, 66 + jh:67 + jh],
                                     xb_v(b)[:, 128 * jh:128 * jh + 128],
                                     q_v, start=True, stop=True)
                srb_f[b] = setup.tile([K // 2, 2], FP32, tag="srb_f",
                                      name=f"srb_f{b}")
                nc.vector.tensor_copy(srb_f[b][:], ps_mix_t[b][:, 66:68])

            def scv(b, jh):
                return ps_sc[b][:, jh * K:(jh + 1) * K]

            def emit_gen_open(b):
                ps_sc[b] = psS.tile([K // 2, 2 * K], FP32, tag="ps_sc",
                                    name=f"ps_sc{b}")
                for jh in range(2):
                    nc.tensor.matmul(scv(b, jh), ident_v, bkt_v(jh),
                                     start=True, stop=False,
                                     skip_group_check=True)

            def emit_gen(b, i0, i1):
                for i in range(i0, i1):
                    eng = PATTERN[i % 32]
                    bias_c = lt_f[b][:, i:i + 1]
                    tj = trelu.tile([E, K], BF16, tag="tj")
                    if eng == "A":
                        nc.scalar.activation(
                            tj[:], rtb_b[b][:], AF.Relu, bias=bias_c,
                            scale=1.0)
                    elif eng == "P":
                        nc.gpsimd.tensor_scalar(
                            out=tj[:], in0=rtb_b[b][:],
                            scalar1=bias_c, scalar2=0.0,
                            op0=ALU.add, op1=ALU.max)
                    else:
                        nc.vector.tensor_scalar(
                            out=tj[:], in0=rtb_b[b][:],
                            scalar1=bias_c, scalar2=0.0,
                            op0=ALU.add, op1=ALU.max)
                    for jh in range(2):
                        nc.tensor.matmul(
                            scv(b, jh)[:, i:i + 1],
                            tj[:, 128 * jh:128 * jh + 128], sgn_v,
                            start=False, stop=(i == K - 1),
                            skip_group_check=True)

            def emit_exp(b):
                for jh in range(2):
                    pT[b][jh] = etiles.tile([K // 2, K], BF16, tag=f"pT{jh}",
                                            name=f"pT{b}_{jh}")
                    nc.scalar.activation(pT[b][jh][:], scv(b, jh),
                                         AF.Exp,
                                         bias=srb_f[b][:, jh:jh + 1],
                                         scale=1.0)

            def emit_attn(b):
                ps_att1 = psN.tile([K // 2, Wn + 1], FP32, tag="ps_att1",
                                   name=f"ps_att1{b}")
                nums = [ps_mix_t[b][:, 0:Wn + 1], ps_att1]
                for ih in range(2):
                    for jh in range(2):
                        nc.tensor.matmul(
                            nums[ih], pT[b][jh][:, 128 * ih:128 * ih + 128],
                            xto_v(b, jh), start=(jh == 0), stop=(jh == 1))
                    rcol = small.tile([K // 2, 1], FP32, tag=f"rcol{ih}",
                                      name=f"rcol{b}_{ih}")
                    nc.vector.reciprocal(rcol[:], nums[ih][:, Wn:Wn + 1])
                    tt[b][ih] = small.tile([K // 2, Wn], BF16, tag=f"tt{ih}",
                                           name=f"tt{b}_{ih}")
                    nc.scalar.activation(tt[b][ih][:], nums[ih][:, 0:Wn],
                                         AF.Tanh, scale=rcol[:])

            def emit_fc_mm(b):
                ps_o = ps_mix_t[b][0:O, 68:68 + Wn]
                for ih in range(2):
                    nc.tensor.matmul(ps_o, fcw_v(ih), tt[b][ih][:],
                                     start=(ih == 0), stop=(ih == 1))

            def emit_fc_out(b):
                nc.scalar.activation(ot2[:, Wn * b:Wn * (b + 1)],
                                     ps_mix_t[b][0:O, 68:68 + Wn],
                                     AF.Identity, bias=fcb_v)

            # software-pipelined emission
            emit_proj(0)
            emit_gen_open(0)
            emit_gen(0, 0, 96)
            emit_proj(1)
            emit_gen(0, 96, K)
            emit_exp(0)
            emit_gen_open(1)
            emit_gen(1, 0, 128)
            emit_attn(0)
            emit_fc_mm(0)
            emit_gen(1, 128, K)
            emit_exp(1)
            emit_fc_out(0)
            nc.sync.dma_start(out=d_out.ap()[:, 0:Wn], in_=ot2[:, 0:Wn])
            emit_attn(1)
            emit_fc_mm(1)
            emit_fc_out(1)
            nc.sync.dma_start(out=d_out.ap()[:, Wn:2 * Wn],
                              in_=ot2[:, Wn:2 * Wn])

    nc.compile()
    return nc


_NC_CACHE = {}


def _get_program():
    if "nc" not in _NC_CACHE:
        _NC_CACHE["nc"] = _build_program()
    return _NC_CACHE["nc"]


def _host_prep(x, lin_w, lin_b, a, bias_kk, fc_w, fc_b):
    f32 = np.float32
    bf16 = ml_dtypes.bfloat16
    x = np.ascontiguousarray(x, f32)
    aa = (np.abs(a) * (1.0 - ALPHA)).astype(f32)
    sgn = np.sign(a).astype(f32)
    alpha_p = ALPHA / (1.0 - ALPHA)

    w1at = (lin_w[:, :Wn] * aa[:, None]).T.astype(f32)          # [64,128]
    w2t = (lin_w[:, Wn:] * aa[:, None]).T                        # [64,128]
    bt = (lin_b * aa)[None, :]
    w2bt = np.concatenate([w2t, bt], 0).astype(f32)              # [65,128]
    q = (alpha_p * (w2bt @ sgn)).astype(f32)                     # [65]
    bkkt = bias_kk.T.astype(f32)                                 # [256,256]
    fcw2t = (0.5 * fc_w).T.astype(f32)                           # [256,64]
    fcb2 = (fc_b + 0.5 * fc_w.sum(1)).astype(f32)                # [64]

    packA = np.zeros((128, A_COLS), f32)
    packA[0:Wn, A_W1:A_W1 + E] = w1at
    packA[0:Wn + 1, A_W2:A_W2 + E] = w2bt
    packA[0:Wn + 1, A_Q] = q

    packB_shared = np.zeros((128, B_COLS), f32)
    packB_shared[:, B_SGN] = sgn
    packB_shared[:, B_BKT:B_BKT + K] = bkkt[0:128, :]
    packB_shared[:, B_BKT + K:B_BKT + 2 * K] = bkkt[128:256, :]
    packB_shared[:, B_ID:B_ID + E] = np.eye(128, dtype=f32)
    packB_shared[:, B_FCW:B_FCW + O] = fcw2t[0:128, :]
    packB_shared[:, B_FCW + O:B_FCW + 2 * O] = fcw2t[128:256, :]
    packB_shared[0:O, B_FCB] = fcb2

    in_maps = []
    for c in range(N_CORES):
        pa = packA.copy()
        pb = packB_shared.copy()
        for i in range(BPC):
            xb = x[BPC * c + i]                                  # [64,256]
            xb1 = np.concatenate([xb, np.ones((1, K), f32)], 0)  # [65,256]
            vt = xb.T                                            # [256,64]
            xto2 = np.concatenate([vt, np.full((K, 1), 2.0, f32)], 1)
            col = A_XB0 if i == 0 else A_XB1
            pa[0:Wn + 1, col:col + K] = xb1
            for h in range(2):
                c0 = B_XTO + (2 * i + h) * (Wn + 1)
                pb[:, c0:c0 + Wn + 1] = xto2[128 * h:128 * h + 128, :]
        in_maps.append({
            "packA": np.ascontiguousarray(pa.astype(bf16)),
            "packB": np.ascontiguousarray(pb.astype(bf16)),
        })
    return in_maps


def kernel(x, lin_w, lin_b, a, bias_kk, fc_w, fc_b, _trace=False):
    nc = _get_program()
    in_maps = _host_prep(np.asarray(x), np.asarray(lin_w), np.asarray(lin_b),
                         np.asarray(a), np.asarray(bias_kk),
                         np.asarray(fc_w), np.asarray(fc_b))
    res = run_bass_kernel_spmd(nc, in_maps, list(range(N_CORES)),
                               trace=_trace)
    out = np.empty((B, Wn, O), np.float32)
    for c in range(N_CORES):
        o = res.results[c]["outp"]          # (O, BPC*Wn)
        for i in range(BPC):
            out[BPC * c + i] = o[:, Wn * i:Wn * (i + 1)].T
    if _trace:
        return out, res
    return out


# revision 15
# speedup vs baseline: 1.2151x; 1.0076x over previous
"""Trainium2 Bass kernel for nn_GatFeatDecoder (GAT-style decoder).

Reference computation per batch b (B=16, W=64, K=256, E=128, O=64):
    v = x[b].T                               (K, W)
    l = v @ W1.T ; r = v @ W2.T              (K, E) each
    e[i,j]  = sum_e a_e * LeakyReLU(l[i,e] + r[j,e] + lin_b[e]) + bias_kk[i,j]
    attn    = softmax_j(e)
    h       = sigmoid(attn @ v)              (K, W)
    out[b]  = h.T @ fc_w.T + fc_b            (W, O)

Data-parallel: 2 batches per core on 8 cores, no collectives.

Math folding (per-core), same as the v1 kernel:
  * z~ = (1-a)|a_e| (l+r+b); sum_e a_e LeakyReLU = sum_e sgn_e relu(z~)
    + alpha' * sum_e sgn_e z~, alpha' = alpha/(1-alpha).  The per-i part
    of the linear term cancels in softmax; the per-j part srb_j =
    sum_w q_w xb[w,j] with q = alpha' * W2b @ sgn precomputed on device,
    and enters as the per-partition bias of the exp() activation.
  * bias_kk^T is accumulated into each score PSUM tile by one extra
    matmul with identity weights, so exp() reads PSUM directly.
  * softmax without row-max (logits bounded): P^T = exp(S^T + srb_j).
    attn@v and the denominator come from matmuls with rhs = [v | 2.0];
    h = sigmoid(num/den) = 0.5*(tanh(num * (0.5/den)) + 1) via ACT Tanh
    with per-partition scale = reciprocal(2*sum exp); the 0.5/0.5 affine
    is folded into the fc weights/bias on the host.

v2 score-matmul inversion (the big change vs v1):
  * v1 streamed each relu tile [E,K] through the PE as the MOVING
    operand of a sign-weighted matmul (256 rows -> ~107ns each, a
    54.6us PE floor for 512 tiles).  v2 makes the relu tile the
    STATIONARY operand instead: per query-node i the tile
    T^i[e, j] = relu(rtb[e,j] + lt[e,i]) is produced once, and two
    matmuls (one per j-half chain) contract it against a single sgn
    column as the moving operand, writing one PSUM column
    S^T[jh][:, i].  Output free size is 1, so each matmul costs ~4ns;
    the whole score reduction is ~2us of PE time and the kernel is
    bound by relu-tile PRODUCTION on DVE/ACT/Pool instead
    (134/391/429 ns per [128,256] tile; DVE runs in 4x mode on bf16).
  * Tiles are indexed by i (bias = lt column) rather than j (bias =
    rtb column) so S^T lands in the same [j-half, i] layout v1 used;
    the exp/attn/fc epilogue is unchanged.
  * The PE is now nearly idle, so the p-state ramp is irrelevant and
    v1's warm-up matmul prologue is dropped entirely.
  * Production is split DVE/ACT/Pool ~20/6/6 per 32 tiles (inverse to
    the measured per-tile costs), with the fixed per-engine work (exp
    on ACT, copies on DVE/Pool) folded into the balance.
"""

import numpy as np
import ml_dtypes

import concourse.bass as bass
import concourse.bacc as bacc
import concourse.tile as tile
from concourse import mybir
from concourse.bass_utils import run_bass_kernel_spmd

ALPHA = 0.2
B, Wn, K, E, O = 16, 64, 256, 128, 64
N_CORES = 8
BPC = B // N_CORES  # batches per core

FP32 = mybir.dt.float32
BF16 = mybir.dt.bfloat16
AF = mybir.ActivationFunctionType
ALU = mybir.AluOpType

# ---- packed-constant column layout (bf16, 128 partitions) ----
# pack A (early: needed for projections + first relu tiles)
A_W1 = 0                      # w1at   [64,128]  rows 0:64
A_W2 = A_W1 + E               # w2bt   [65,128]  rows 0:65
A_Q = A_W2 + E                # q      [65,1]
A_XB0 = A_Q + 1               # xb b0  [65,256]  rows 0:65 (row 64 = ones)
A_XB1 = A_XB0 + K             # xb b1  [65,256]
A_COLS = A_XB1 + K

# pack B (epilogue constants; lands while batch-0 tiles stream)
B_SGN = 0                     # sgn column [128,1]
B_XTO = B_SGN + 1             # xto2   4 x [128,65]  (b,h) = [v | 2.0]
B_BKT = B_XTO + 4 * (Wn + 1)  # bkkt^T 2 x [128,256]
B_ID = B_BKT + 2 * K          # identity [128,128]
B_FCW = B_ID + E              # fcw2t  2 x [128,64]
B_FCB = B_FCW + 2 * O         # fcb2   [64,1]
B_COLS = B_FCB + 1

# tile-production engine split per 32 i's: DVE 20, ACT 6, Pool 6
# (largest-remainder interleave of the measured per-tile costs)
def _make_pattern(n_v=20, n_a=6, n_p=6):
    quota = {"V": n_v / 32.0, "A": n_a / 32.0, "P": n_p / 32.0}
    acc = {"V": 0.0, "A": 0.0, "P": 0.0}
    pat = []
    for _ in range(32):
        for k in acc:
            acc[k] += quota[k]
        k = max(acc, key=lambda t: acc[t])
        acc[k] -= 1.0
        pat.append(k)
    return pat

PATTERN = _make_pattern()


def _build_program():
    nc = bacc.Bacc("TRN2", target_bir_lowering=False, debug=False,
                   num_devices=N_CORES)

    d_packA = nc.dram_tensor("packA", [128, A_COLS], BF16, kind="ExternalInput")
    d_packB = nc.dram_tensor("packB", [128, B_COLS], BF16, kind="ExternalInput")
    d_out = nc.dram_tensor("outp", [O, BPC * Wn], FP32, kind="ExternalOutput")

    with tile.TileContext(nc) as tc:
        with (
            tc.tile_pool(name="consts", bufs=1) as consts,
            tc.tile_pool(name="setup", bufs=2) as setup,
            tc.tile_pool(name="trelu", bufs=64) as trelu,
            tc.tile_pool(name="etiles", bufs=4) as etiles,
            tc.tile_pool(name="small", bufs=8) as small,
            tc.tile_pool(name="psA", bufs=2, space="PSUM") as psA,
            tc.tile_pool(name="psS", bufs=2, space="PSUM") as psS,
            tc.tile_pool(name="psM", bufs=2, space="PSUM") as psM,
            tc.tile_pool(name="psN", bufs=2, space="PSUM") as psN,
        ):
            cA = consts.tile([128, A_COLS], BF16, tag="cA")
            # chunk 1: weights + q + xb0 (gates batch-0 projections);
            # chunk 2: xb1
            nc.sync.dma_start(out=cA[:, 0:A_XB1], in_=d_packA.ap()[:, 0:A_XB1])
            nc.sync.dma_start(out=cA[:, A_XB1:A_COLS],
                              in_=d_packA.ap()[:, A_XB1:A_COLS])
            cB = consts.tile([128, B_COLS], BF16, tag="cB")
            nc.gpsimd.dma_start(out=cB[:], in_=d_packB.ap())

            w1at_v = cA[0:Wn, A_W1:A_W1 + E]
            w2bt_v = cA[0:Wn + 1, A_W2:A_W2 + E]
            q_v = cA[0:Wn + 1, A_Q:A_Q + 1]

            def xb_v(b):
                c = A_XB0 if b == 0 else A_XB1
                return cA[0:Wn + 1, c:c + K]

            sgn_v = cB[:, B_SGN:B_SGN + 1]

            def xto_v(b, h):
                c = B_XTO + (2 * b + h) * (Wn + 1)
                return cB[:, c:c + Wn + 1]

            def bkt_v(jh):
                return cB[:, B_BKT + jh * K:B_BKT + (jh + 1) * K]

            ident_v = cB[:, B_ID:B_ID + E]

            def fcw_v(ih):
                return cB[:, B_FCW + ih * O:B_FCW + (ih + 1) * O]

            fcb_v = cB[0:O, B_FCB:B_FCB + 1]

            # per-batch state
            lt_f = [None] * BPC     # fp32 [E,K]  (scalar/bias source, per-i)
            rtb_b = [None] * BPC    # bf16 [E,K]  (tile in0)
            srb_f = [None] * BPC
            ps_sc = [None] * BPC    # pair of [K//2, K] score PSUM tiles
            pT = [[None, None] for _ in range(BPC)]
            tt = [[None, None] for _ in range(BPC)]
            ps_mix_t = [None] * BPC  # [:,0:65] attn ih0 | [:,66:68] srb | [0:64,68:132] fc

            ot2 = consts.tile([O, BPC * Wn], FP32, tag="ot2")

            def emit_proj(b):
                ps_lr = psA.tile([E, 2 * K], FP32, tag="ps_lr",
                                 name=f"ps_lr{b}")
                # rtb matmul first so its copy (which gates every tile)
                # starts while the lt matmul still runs
                nc.tensor.matmul(ps_lr[:, K:2 * K], w2bt_v, xb_v(b),
                                 start=True, stop=True)
                nc.tensor.matmul(ps_lr[:, 0:K], w1at_v, xb_v(b)[0:Wn, :],
                                 start=True, stop=True)
                lt_f[b] = setup.tile([E, K], FP32, tag="lt_f", name=f"lt_f{b}")
                rtb_b[b] = setup.tile([E, K], BF16, tag="rtb_b",
                                      name=f"rtb_b{b}")
                # both copies on DVE: rtb feeds the first DVE tiles
                # in-order; Pool cannot read PSUM
                nc.vector.tensor_copy(rtb_b[b][:], ps_lr[:, K:2 * K])
                nc.vector.tensor_copy(lt_f[b][:], ps_lr[:, 0:K])
                # srb columns: srb[j] = sum_w q[w] xb[w, j]
                ps_mix_t[b] = psM.tile([128, 132], FP32, tag="ps_mix",
                                       name=f"ps_mix{b}")
                for jh in range(2):
                    nc.tensor.matmul(ps_mix_t[b][:, 66 + jh:67 + jh],
                                     xb_v(b)[:, 128 * jh:128 * jh + 128],
                                     q_v, start=True, stop=True)
                srb_f[b] = setup.tile([K // 2, 2], FP32, tag="srb_f",
                                      name=f"srb_f{b}")
                nc.vector.tensor_copy(srb_f[b][:], ps_mix_t[b][:, 66:68])

            def scv(b, jh):
                return ps_sc[b][:, jh * K:(jh + 1) * K]

            def emit_gen_open(b):
                ps_sc[b] = psS.tile([K // 2, 2 * K], FP32, tag="ps_sc",
                                    name=f"ps_sc{b}")
                for jh in range(2):
                    nc.tensor.matmul(scv(b, jh), ident_v, bkt_v(jh),
                                     start=True, stop=False,
                                     skip_group_check=True)

            def emit_gen(b, i0, i1):
                for i in range(i0, i1):
                    eng = PATTERN[i % 32]
                    bias_c = lt_f[b][:, i:i + 1]
                    tj = trelu.tile([E, K], BF16, tag="tj")
                    if eng == "A":
                        nc.scalar.activation(
                            tj[:], rtb_b[b][:], AF.Relu, bias=bias_c,
                            scale=1.0)
                    elif eng == "P":
                        nc.gpsimd.tensor_scalar(
                            out=tj[:], in0=rtb_b[b][:],
                            scalar1=bias_c, scalar2=0.0,
                            op0=ALU.add, op1=ALU.max)
                    else:
                        nc.vector.tensor_scalar(
                            out=tj[:], in0=rtb_b[b][:],
                            scalar1=bias_c, scalar2=0.0,
                            op0=ALU.add, op1=ALU.max)
                    for jh in range(2):
                        nc.tensor.matmul(
                            scv(b, jh)[:, i:i + 1],
                            tj[:, 128 * jh:128 * jh + 128], sgn_v,
                            start=False, stop=(i == K - 1),
                            skip_group_check=True)

            def emit_exp_chunk(b, ih):
                # columns [128*ih, 128*ih+128) of both jh chains are final
                # as soon as their col-matmuls have landed; chunked exp
                # overlaps the softmax with later tile production
                for jh in range(2):
                    if pT[b][jh] is None:
                        pT[b][jh] = etiles.tile([K // 2, K], BF16,
                                                tag=f"pT{jh}",
                                                name=f"pT{b}_{jh}")
                    c0, c1 = 128 * ih, 128 * ih + 128
                    nc.scalar.activation(pT[b][jh][:, c0:c1],
                                         scv(b, jh)[:, c0:c1], AF.Exp,
                                         bias=srb_f[b][:, jh:jh + 1],
                                         scale=1.0)

            def emit_attn_ih(b, ih):
                if ih == 0:
                    num = ps_mix_t[b][:, 0:Wn + 1]
                else:
                    num = psN.tile([K // 2, Wn + 1], FP32, tag="ps_att1",
                                   name=f"ps_att1{b}")
                for jh in range(2):
                    nc.tensor.matmul(
                        num, pT[b][jh][:, 128 * ih:128 * ih + 128],
                        xto_v(b, jh), start=(jh == 0), stop=(jh == 1))
                rcol = small.tile([K // 2, 1], FP32, tag=f"rcol{ih}",
                                  name=f"rcol{b}_{ih}")
                nc.vector.reciprocal(rcol[:], num[:, Wn:Wn + 1])
                tt[b][ih] = small.tile([K // 2, Wn], BF16, tag=f"tt{ih}",
                                       name=f"tt{b}_{ih}")
                nc.scalar.activation(tt[b][ih][:], num[:, 0:Wn],
                                     AF.Tanh, scale=rcol[:])

            def emit_fc_mm(b):
                ps_o = ps_mix_t[b][0:O, 68:68 + Wn]
                for ih in range(2):
                    nc.tensor.matmul(ps_o, fcw_v(ih), tt[b][ih][:],
                                     start=(ih == 0), stop=(ih == 1))

            def emit_fc_out(b):
                nc.scalar.activation(ot2[:, Wn * b:Wn * (b + 1)],
                                     ps_mix_t[b][0:O, 68:68 + Wn],
                                     AF.Identity, bias=fcb_v)

            # software-pipelined emission: epilogue pieces are interleaved
            # into the tile stream so in-order engine queues never block on
            # a not-yet-ready epilogue instruction (emission position is
            # queue position), and only a short chain trails the last tile
            emit_proj(0)
            emit_gen_open(0)
            emit_gen(0, 0, 96)
            emit_proj(1)
            emit_gen(0, 96, 144)
            emit_exp_chunk(0, 0)        # b0 cols [0:128) (done by tile 127)
            emit_gen(0, 144, 160)
            emit_attn_ih(0, 0)
            emit_gen(0, 160, K)
            emit_gen_open(1)
            emit_gen(1, 0, 40)
            emit_exp_chunk(0, 1)        # b0 cols [128:256)
            emit_gen(1, 40, 56)
            emit_attn_ih(0, 1)
            emit_gen(1, 56, 72)
            emit_fc_mm(0)
            emit_fc_out(0)
            nc.sync.dma_start(out=d_out.ap()[:, 0:Wn], in_=ot2[:, 0:Wn])
            emit_gen(1, 72, 144)
            emit_exp_chunk(1, 0)        # b1 cols [0:128)
            emit_gen(1, 144, 160)
            emit_attn_ih(1, 0)
            emit_gen(1, 160, K)
            emit_exp_chunk(1, 1)        # b1 cols [128:256)
            emit_attn_ih(1, 1)
            emit_fc_mm(1)
            emit_fc_out(1)
            nc.sync.dma_start(out=d_out.ap()[:, Wn:2 * Wn],
                              in_=ot2[:, Wn:2 * Wn])

    nc.compile()
    return nc


_NC_CACHE = {}


def _get_program():
    if "nc" not in _NC_CACHE:
        _NC_CACHE["nc"] = _build_program()
    return _NC_CACHE["nc"]


def _host_prep(x, lin_w, lin_b, a, bias_kk, fc_w, fc_b):
    f32 = np.float32
    bf16 = ml_dtypes.bfloat16
    x = np.ascontiguousarray(x, f32)
    aa = (np.abs(a) * (1.0 - ALPHA)).astype(f32)
    sgn = np.sign(a).astype(f32)
    alpha_p = ALPHA / (1.0 - ALPHA)

    w1at = (lin_w[:, :Wn] * aa[:, None]).T.astype(f32)          # [64,128]
    w2t = (lin_w[:, Wn:] * aa[:, None]).T                        # [64,128]
    bt = (lin_b * aa)[None, :]
    w2bt = np.concatenate([w2t, bt], 0).astype(f32)              # [65,128]
    q = (alpha_p * (w2bt @ sgn)).astype(f32)                     # [65]
    bkkt = bias_kk.T.astype(f32)                                 # [256,256]
    fcw2t = (0.5 * fc_w).T.astype(f32)                           # [256,64]
    fcb2 = (fc_b + 0.5 * fc_w.sum(1)).astype(f32)                # [64]

    packA = np.zeros((128, A_COLS), f32)
    packA[0:Wn, A_W1:A_W1 + E] = w1at
    packA[0:Wn + 1, A_W2:A_W2 + E] = w2bt
    packA[0:Wn + 1, A_Q] = q

    packB_shared = np.zeros((128, B_COLS), f32)
    packB_shared[:, B_SGN] = sgn
    packB_shared[:, B_BKT:B_BKT + K] = bkkt[0:128, :]
    packB_shared[:, B_BKT + K:B_BKT + 2 * K] = bkkt[128:256, :]
    packB_shared[:, B_ID:B_ID + E] = np.eye(128, dtype=f32)
    packB_shared[:, B_FCW:B_FCW + O] = fcw2t[0:128, :]
    packB_shared[:, B_FCW + O:B_FCW + 2 * O] = fcw2t[128:256, :]
    packB_shared[0:O, B_FCB] = fcb2

    in_maps = []
    for c in range(N_CORES):
        pa = packA.copy()
        pb = packB_shared.copy()
        for i in range(BPC):
            xb = x[BPC * c + i]                                  # [64,256]
            xb1 = np.concatenate([xb, np.ones((1, K), f32)], 0)  # [65,256]
            vt = xb.T                                            # [256,64]
            xto2 = np.concatenate([vt, np.full((K, 1), 2.0, f32)], 1)
            col = A_XB0 if i == 0 else A_XB1
            pa[0:Wn + 1, col:col + K] = xb1
            for h in range(2):
                c0 = B_XTO + (2 * i + h) * (Wn + 1)
                pb[:, c0:c0 + Wn + 1] = xto2[128 * h:128 * h + 128, :]
        in_maps.append({
            "packA": np.ascontiguousarray(pa.astype(bf16)),
            "packB": np.ascontiguousarray(pb.astype(bf16)),
        })
    return in_maps


def kernel(x, lin_w, lin_b, a, bias_kk, fc_w, fc_b, _trace=False):
    nc = _get_program()
    in_maps = _host_prep(np.asarray(x), np.asarray(lin_w), np.asarray(lin_b),
                         np.asarray(a), np.asarray(bias_kk),
                         np.asarray(fc_w), np.asarray(fc_b))
    res = run_bass_kernel_spmd(nc, in_maps, list(range(N_CORES)),
                               trace=_trace)
    out = np.empty((B, Wn, O), np.float32)
    for c in range(N_CORES):
        o = res.results[c]["outp"]          # (O, BPC*Wn)
        for i in range(BPC):
            out[BPC * c + i] = o[:, Wn * i:Wn * (i + 1)].T
    if _trace:
        return out, res
    return out
